# revision 10
# baseline (speedup 1.0000x reference)
"""Causal self-attention with RoPE on 8 Trainium2 NeuronCores.

Reference computation (B=4, T=2048, D=1024, H=16, hd=64, fp32):
    qkv = x @ w_qkv ; q,k per-head RoPE (interleaved pairs) ;
    out = softmax(causal(q k^T / 8)) @ v ; out @ w_proj

Sharding: core c -> (batch b = c//2, head-group g = c%2 of 8 heads).
Data parallel on B, tensor parallel on heads; w_proj is row-parallel so each
core returns a partial [2048, 1024] product and the host sums the two
partials per batch (the "all-reduce" of the row-parallel linear).

Per-core device program (all matmuls in fp32r: fp32 with 11-bit mantissa
round, full PE speed at moving-dim >= 256; accumulation fp32 in PSUM):

  Phase A+B (fused with the input stream): x^T arrives in [kt, 512-token]
  chunks interleaved with the w_v chunks so the v = x@w_v matmuls run
  behind the DMA stream; w_qk / cos/sin chunks follow so the q/k chains
  start the moment the stream drains.  q,k in transposed layout [feat, t]
  (lhsT = w tiles, rhs = x^T chunks); v in natural layout with a ones
  column appended (softmax denominators fall out of the PV matmul).
  RoPE: the q/k weights are augmented host-side (per-head [evens|odds]
  column blocks) so   q_rot = q*cc + (perm@q)*ss   where perm is a signed
  32-row block swap.  Since the cc/ss tables repeat with row-period 32,
  perm@(q*ss) == (perm@q)*ss, which lets the ss-multiply run on the idle
  GPSIMD/Pool engine from the SBUF copy of q (the PSUM->SBUF copy already
  exists for the perm matmul): DVE does only 2 ops per tile (cc-mult, add).

  Phase C: attention one head at a time in transposed-score layout
  S^T[j, i] (keys on partitions), 1024-query halves (ih).  Per key tile:
  PE scores -> DVE adds -3e38 tri mask on the diagonal block -> ACT exp
  straight into fp32r P^T -> PE accumulates out^T via matmul(lhsT=[v|1]).
  The PV accumulators [65, 1024] ping-pong across heads (2 PSUM banks
  each) so the normalization chain (DVE reciprocal of the ones row, Pool
  partition-broadcast, DVE multiply into SBUF) of head h overlaps head
  h+1's score/exp/PV pipeline; the exp->PV software pipeline is carried
  across head boundaries.  128-wide score matmuls (fp32r 4x penalty) are
  padded down to 256 into never-read masked columns.  The row-parallel
  projection of query-half 0 is interleaved into half 1's ACT-bound
  pipeline (one token tile per head); half 1's projection is the tail.
"""

import numpy as np

import concourse.bass as bass
import concourse.tile as tile
from concourse import bacc, mybir
from concourse.bass_utils import run_bass_kernel_spmd
from neuron_dtypes._impl import fp32r as fp32r_impl

F32 = mybir.dt.float32
F32R = mybir.dt.float32r
AF = mybir.ActivationFunctionType
OP = mybir.AluOpType

B, T, D, NH, HD = 4, 2048, 1024, 16, 64
HPC = 8            # heads per core
NEG = -3.0e38
N_CORES = 8
NTT = T // 128     # 16 token tiles
NKT = D // 128     # 8 contraction tiles

# q/k feature-tile processing order: (q pr, k pr) pairs so attention deps
# resolve earliest for pr 0.
FT_ORDER = [0, 4, 1, 5, 2, 6, 3, 7]


def _round_fp32r(x: np.ndarray) -> np.ndarray:
    xb = np.ascontiguousarray(x).view(np.uint32).ravel()
    r = np.asarray(fp32r_impl.cast_fp32_to_fp32r(len(xb), xb), dtype=np.uint32)
    return r.view(np.float32).reshape(x.shape)


def _seg_list(i0: int, j0: int):
    """Score/PV segments in tile-local columns: [max(i0,j0)-i0, 1024) split
    at the 512 PSUM bank boundary."""
    lo = max(i0, j0) - i0
    segs = []
    while lo < 1024:
        hi = min(1024, (lo // 512 + 1) * 512)
        segs.append((lo, hi))
        lo = hi
    return segs


def _build_program(reps: int = 1, stop_after: str = 'full'):
    nc = bacc.Bacc("TRN2", target_bir_lowering=False, debug=False)
    x_d = nc.dram_tensor("x", [D, T], F32R, kind="ExternalInput")  # x^T, host-transposed
    wqk_d = nc.dram_tensor("wqk", [D, 1024], F32R, kind="ExternalInput")
    perm_d = nc.dram_tensor("perm", [128, 128], F32R, kind="ExternalInput")
    wv_d = nc.dram_tensor("wv", [D, 512], F32R, kind="ExternalInput")
    wp_d = nc.dram_tensor("wproj", [512, D], F32R, kind="ExternalInput")
    cc_d = nc.dram_tensor("cc", [128, T], F32, kind="ExternalInput")
    ss_d = nc.dram_tensor("ss", [128, T], F32, kind="ExternalInput")
    tri_d = nc.dram_tensor("tri", [128, 128], F32, kind="ExternalInput")
    out_d = nc.dram_tensor("out", [T, D], F32, kind="ExternalOutput")

    with tile.TileContext(nc) as tc:
      for _rep in range(reps):
        with (
            tc.tile_pool(name="persist", bufs=1) as pers,
            tc.tile_pool(name="vo", bufs=1) as vop,
            tc.tile_pool(name="qkt", bufs=1) as qktp,
        ):
            tri = pers.tile([128, 128], F32, tag="tri")
            perm = pers.tile([128, 128], F32R, tag="perm")
            nc.sync.dma_start(tri[:], tri_d[:])
            nc.sync.dma_start(perm[:], perm_d[:])

            # [128, h, 65] per token tile: v columns 0:64, ones at col 64
            vo = [vop.tile([128, HPC, 65], F32R, tag=f"vo{tt}", name=f"vo{tt}")
                  for tt in range(NTT)]
            for tt in range(NTT):
                nc.vector.memset(vo[tt][:, :, 64:65].bitcast(F32), 1.0)

            # roped q/k, transposed: tiles 0..3 = q pairs, 4..7 = k pairs
            qkt = [
                qktp.tile([128, T], F32R, tag=f"qkt{i}", name=f"qkt{i}")
                for i in range(8)
            ]

            with (
                tc.tile_pool(name="xt", bufs=1) as xtp,
            ):
                # x^T as per-(kt, 512-token-chunk) tiles
                xt = [
                    [xtp.tile([128, 512], F32R, tag=f"xt{kt}_{c}",
                              name=f"xt{kt}_{c}") for c in range(4)]
                    for kt in range(NKT)
                ]
                wr = wqk_d.rearrange("(t p) f -> p t f", p=128)

                with (
                    tc.tile_pool(name="wvp", bufs=1) as wvp,
                    tc.tile_pool(name="vps", bufs=2, space="PSUM") as vps,
                ):
                    wv_k = [wvp.tile([128, 512], F32R, tag=f"wv{kt}", name=f"wv{kt}")
                            for kt in range(NKT)]

                    # ---- DMA stream, in compute-unlock order ------------
                    for kt in range(NKT):
                        nc.sync.dma_start(wv_k[kt][:],
                                          wv_d[kt * 128:(kt + 1) * 128, :])
                        nc.sync.dma_start(
                            xt[kt][0][:], x_d[kt * 128:(kt + 1) * 128, 0:512])
                    for c in range(1, 4):
                        for kt in range(NKT):
                            nc.sync.dma_start(
                                xt[kt][c][:],
                                x_d[kt * 128:(kt + 1) * 128,
                                    c * 512:(c + 1) * 512])

                    # ---- v (natural layout), streaming behind the DMAs --
                    for tt in range(NTT):
                        c, t_lo = tt // 4, (tt % 4) * 128
                        ps = vps.tile([128, 512], F32, tag="vps")
                        for kt in range(NKT):
                            nc.tensor.matmul(
                                ps[:], xt[kt][c][:, t_lo:t_lo + 128], wv_k[kt][:],
                                start=(kt == 0), stop=(kt == NKT - 1),
                            )
                        nc.scalar.copy(
                            vo[tt][:, :, 0:64],
                            ps[:].rearrange("p (h d) -> p h d", h=HPC),
                        )

                # ---- q/k + rope, 1-unit software pipeline ---------------
                with (
                    tc.tile_pool(name="ccss", bufs=1) as ccssp,
                    tc.tile_pool(name="ftw", bufs=4) as ftwp,
                    tc.tile_pool(name="ropet", bufs=2) as rp,
                    tc.tile_pool(name="qkps", bufs=4, space="PSUM") as qkps,
                ):
                    cc_c = [ccssp.tile([128, 512], F32, tag=f"cc{c}", name=f"cc{c}")
                            for c in range(4)]
                    ss_c = [ccssp.tile([128, 512], F32, tag=f"ss{c}", name=f"ss{c}")
                            for c in range(4)]
                    w_a = {}
                    for c in range(4):
                        nc.sync.dma_start(cc_c[c][:], cc_d[:, c * 512:(c + 1) * 512])
                        nc.sync.dma_start(ss_c[c][:], ss_d[:, c * 512:(c + 1) * 512])
                        for ft in (FT_ORDER[2 * c], FT_ORDER[2 * c + 1]):
                            w_a[ft] = ftwp.tile([128, NKT, 128], F32R, tag="wa",
                                                name=f"wa{ft}")
                            nc.sync.dma_start(
                                w_a[ft][:], wr[:, :, ft * 128:(ft + 1) * 128])
                    def emit_qmm(ft, tcn):
                        ps_a = qkps.tile([128, 512], F32, tag="qkps",
                                         name=f"qk{ft}_{tcn}")
                        for kt in range(NKT):
                            nc.tensor.matmul(
                                ps_a[:], w_a[ft][:, kt, :], xt[kt][tcn][:],
                                start=(kt == 0), stop=(kt == NKT - 1),
                            )
                        q_tmp = rp.tile([128, 512], F32R, tag="qtmp",
                                        name=f"qt{ft}_{tcn}")
                        nc.scalar.copy(q_tmp[:], ps_a[:])
                        q_ss = rp.tile([128, 512], F32R, tag="qss",
                                       name=f"qs{ft}_{tcn}")
                        nc.gpsimd.tensor_tensor(q_ss[:], q_tmp[:], ss_c[tcn][:],
                                                OP.mult)
                        return ps_a, q_ss

                    def emit_rope(ft, tcn, ps_a, q_ss):
                        sl = slice(tcn * 512, (tcn + 1) * 512)
                        ps_b = qkps.tile([128, 512], F32, tag="qkpsb",
                                         name=f"qkb{ft}_{tcn}")
                        nc.tensor.matmul(ps_b[:], perm[:], q_ss[:],
                                         start=True, stop=True)
                        t1 = rp.tile([128, 512], F32, tag="t1")
                        nc.vector.tensor_tensor(t1[:], ps_a[:], cc_c[tcn][:],
                                                OP.mult)
                        nc.vector.tensor_tensor(qkt[ft][:, sl], t1[:], ps_b[:],
                                                OP.add)

                    pending = None
                    for ft in FT_ORDER:
                        for tcn in range(4):
                            cur = emit_qmm(ft, tcn)
                            if pending is not None:
                                emit_rope(pending[0], pending[1], *pending[2])
                            pending = (ft, tcn, cur)
                    emit_rope(pending[0], pending[1], *pending[2])

            if stop_after == 'qkv':
                with tc.tile_pool(name="dump", bufs=2) as dp:
                    for i in range(8):
                        for tcn in range(2):
                            d = dp.tile([128, 1024], F32, tag="d")
                            nc.vector.tensor_copy(
                                d[:], qkt[i][:, tcn * 1024:(tcn + 1) * 1024].bitcast(F32))
                            blk = 2 * i + tcn
                            nc.sync.dma_start(out_d[blk * 128:(blk + 1) * 128, :], d[:])
                continue

            # ---- phase C: attention + projection ------------------------
            with (
                tc.tile_pool(name="wpp", bufs=1) as wpp,
                tc.tile_pool(name="pt", bufs=4) as ptp,
                tc.tile_pool(name="nrm", bufs=2) as nrmp,
                tc.tile_pool(name="osb", bufs=3) as osbp,
                tc.tile_pool(name="stps", bufs=2, space="PSUM") as stps,
                tc.tile_pool(name="atps", bufs=2, space="PSUM") as atps,
                tc.tile_pool(name="atsb", bufs=1) as atsbp,
            ):
                wp_sb = wpp.tile([128, 4, D], F32R, tag="wp")
                nc.sync.dma_start(wp_sb[:], wp_d.rearrange("(t p) f -> p t f", p=128))

                at_tiles = {
                    ih: [atsbp.tile([128, 1024], F32R, tag=f"at{ih}_{pr}",
                                    name=f"at{ih}_{pr}") for pr in range(4)]
                    for ih in range(2)
                }

                def emit_proj_tile(ih2, tl):
                    tt = 8 * ih2 + tl
                    pp = stps.tile([128, 1024], F32, tag="st", name=f"pp{tt}")
                    for nch in range(2):
                        for mt in range(4):
                            nc.tensor.matmul(
                                pp[:, nch * 512:(nch + 1) * 512],
                                at_tiles[ih2][mt][:, tl * 128:(tl + 1) * 128],
                                wp_sb[:, mt, nch * 512:(nch + 1) * 512],
                                start=(mt == 0), stop=(mt == 3),
                            )
                    o_sb = osbp.tile([128, 1024], F32, tag="osb")
                    nc.vector.tensor_copy(o_sb[:], pp[:])
                    nc.sync.dma_start(out_d[tt * 128:(tt + 1) * 128, :], o_sb[:])

                # pv software pipeline carried across heads, lag 2 so the
                # exp -> Pool-trim -> pv chain never blocks the PE stream
                pv_q = []           # [(h, jt, pt, at_ps, i0, n_jt, i_lo_loc)]
                norm_q = []         # [(ih, h, at_ps, age)]

                def emit_pv(pv):
                    h, jt, pt, at_ps, i0, n_jt, i_lo_loc = pv
                    j0 = 128 * jt
                    for (lo, hi) in _seg_list(i0, j0):
                        last_jt = min(n_jt - 1, (i0 + hi - 1) // 128)
                        nc.tensor.matmul(
                            at_ps[:, lo:hi],
                            vo[jt][:, h, :],
                            pt[:, lo - i_lo_loc:hi - i_lo_loc],
                            start=(jt == 0), stop=(jt == last_jt),
                        )

                def emit_norm(nrm):
                    ih2, h2, at_ps = nrm
                    pr2, r02 = h2 // 2, 64 * (h2 % 2)
                    sum_sb = nrmp.tile([1, 1024], F32, tag="sum")
                    r_sb = nrmp.tile([1, 1024], F32, tag="r")
                    rb_sb = nrmp.tile([64, 1024], F32, tag="rb")
                    nc.vector.tensor_copy(sum_sb[:], at_ps[64:65, :])
                    nc.vector.reciprocal_approx_fast(r_sb[:], sum_sb[:])
                    nc.gpsimd.partition_broadcast(rb_sb[:], r_sb[:])
                    nc.vector.tensor_tensor(
                        at_tiles[ih2][pr2][r02:r02 + 64, :], at_ps[0:64, :],
                        rb_sb[:], OP.mult,
                    )

                for ih in range(2):
                    i0 = 1024 * ih
                    n_jt = 8 * ih + 8
                    for h in range(HPC):
                        pr, r0 = h // 2, 64 * (h % 2)
                        qt_ap = qkt[pr][r0:r0 + 64, :]
                        kt_ap = qkt[4 + pr][r0:r0 + 64, :]
                        at_ps = atps.tile([65, 1024], F32, tag="atps",
                                          name=f"at{ih}_{h}")
                        for jt in range(n_jt):
                            j0 = 128 * jt
                            i_lo_loc = max(i0, j0) - i0
                            st = stps.tile([128, 1024], F32, tag="st",
                                           name=f"st{ih}_{h}_{jt}")
                            for (lo, hi) in _seg_list(i0, j0):
                                # fp32r matmuls narrower than 256 run 4x
                                # slower: widen into masked, never-read cols
                                wlo = lo if hi - lo >= 256 else max(
                                    (lo // 512) * 512, hi - 256)
                                nc.tensor.matmul(
                                    st[:, wlo:hi],
                                    kt_ap[:, j0:j0 + 128],
                                    qt_ap[:, i0 + wlo:i0 + hi],
                                    start=True, stop=True,
                                )
                            pt = ptp.tile([128, 1024], F32R, tag="pt",
                                          name=f"pt{ih}_{h}_{jt}")
                            nc.scalar.activation(
                                pt[:, 0:1024 - i_lo_loc], st[:, i_lo_loc:],
                                AF.Exp,
                            )
                            if j0 >= i0:
                                # zero the masked upper-triangle of the diag
                                # block (always pt cols 0:128) on Pool --
                                # keeps the st->exp chain free of DVE/Pool
                                nc.gpsimd.tensor_tensor(
                                    pt[:, 0:128], pt[:, 0:128], tri[:],
                                    OP.mult,
                                )
                            if len(pv_q) >= 2:
                                emit_pv(pv_q.pop(0))
                            pv_q.append((h, jt, pt, at_ps, i0, n_jt, i_lo_loc))
                            # fill ACT-bound slack in half 1 with the half-0
                            # projection (one token tile per head, early)
                            if (stop_after == 'full' and ih == 1 and jt == 2
                                    and h >= 1):
                                emit_proj_tile(0, h - 1)
                        norm_q.append((ih, h, at_ps, 0))
                        # norm of the previous head, emitted at this head's
                        # end: its Pool broadcast lands in the gap between
                        # trim bursts instead of stalling them
                        while norm_q and not any(
                                p[3] is norm_q[0][2] for p in pv_q):
                            emit_norm(norm_q.pop(0)[:3])

                    if stop_after == 'attn':
                        while pv_q:
                            emit_pv(pv_q.pop(0))
                        while norm_q:
                            emit_norm(norm_q.pop(0)[:3])
                        for pr in range(4):
                            dmp = osbp.tile([128, 1024], F32, tag="osb")
                            nc.vector.tensor_copy(
                                dmp[:], at_tiles[ih][pr][:].bitcast(F32))
                            nc.sync.dma_start(
                                out_d[(8 * ih + 2 * pr) * 128:
                                      (8 * ih + 2 * pr + 1) * 128, :], dmp[:])

                if stop_after == 'full':
                    while pv_q:
                        emit_pv(pv_q.pop(0))
                    while norm_q:
                        emit_norm(norm_q.pop(0)[:3])
                    emit_proj_tile(0, 7)
                    for tl in range(8):
                        emit_proj_tile(1, tl)
    nc.compile()
    return nc


_NC_CACHE = None


def _get_program():
    global _NC_CACHE
    if _NC_CACHE is None:
        _NC_CACHE = _build_program()
    return _NC_CACHE


def _host_inputs(x, cos, sin, w_qkv, w_proj):
    """Build the 8 per-core input dicts."""
    x = np.asarray(x, np.float32)
    cos = np.asarray(cos, np.float32)
    sin = np.asarray(sin, np.float32)
    w_qkv = np.asarray(w_qkv, np.float32)
    w_proj = np.asarray(w_proj, np.float32)

    cct = np.tile(cos.T, (4, 1)).astype(np.float32)          # [128, T]
    sst = np.tile(sin.T, (4, 1)).astype(np.float32)
    # multiplicative causal mask for the diagonal block: P^T[j, i] *= (i >= j)
    tri = (np.arange(128)[None, :] >= np.arange(128)[:, None]).astype(np.float32)
    # signed 32-row-block swap (as matmul lhsT): out[m] = s(m) * in[src(m)]
    perm = np.zeros((128, 128), np.float32)
    for m in range(128):
        if m % 64 < 32:
            perm[m + 32, m] = -1.0
        else:
            perm[m - 32, m] = 1.0

    x_r = [_round_fp32r(np.ascontiguousarray(x[b].T)) for b in range(B)]

    wq = w_qkv[:, 0:D]
    wk = w_qkv[:, D:2 * D] * np.float32(1.0 / np.sqrt(HD))
    wv = w_qkv[:, 2 * D:3 * D]

    def build_qk_aug(g):
        cols = []
        # ft 0..3: q pairs, ft 4..7: k pairs (evens then odds per head)
        for w in (wq, wk):
            for pr in range(4):
                for hl in (0, 1):
                    hw = w[:, (g * 8 + 2 * pr + hl) * 64:(g * 8 + 2 * pr + hl + 1) * 64]
                    ev, od = hw[:, 0::2], hw[:, 1::2]
                    cols.append(np.concatenate([ev, od], axis=1))
        return np.concatenate(cols, axis=1)  # [D, 1024]

    wqk_g = [_round_fp32r(build_qk_aug(g)) for g in range(2)]
    wv_g = [_round_fp32r(np.ascontiguousarray(wv[:, g * 512:(g + 1) * 512]))
            for g in range(2)]
    wp_g = [_round_fp32r(np.ascontiguousarray(w_proj[g * 512:(g + 1) * 512, :]))
            for g in range(2)]

    in_maps = []
    for c in range(N_CORES):
        b, g = c // 2, c % 2
        in_maps.append({
            "x": x_r[b], "wqk": wqk_g[g], "wv": wv_g[g], "wproj": wp_g[g],
            "cc": cct, "ss": sst, "tri": tri,
            "perm": _round_fp32r(perm),
        })
    return in_maps


def kernel(x, cos, sin, w_qkv, w_proj):
    nc = _get_program()
    in_maps = _host_inputs(x, cos, sin, w_qkv, w_proj)
    res = run_bass_kernel_spmd(nc, in_maps, core_ids=list(range(N_CORES)))
    out = np.empty((B, T, D), dtype=np.float32)
    for b in range(B):
        out[b] = res.results[2 * b]["out"] + res.results[2 * b + 1]["out"]
    return out
